# revision 23
# baseline (speedup 1.0000x reference)
"""Multi-head attention Trainium2 kernel, 8-core SPMD.

Sharding: core = (batch b = core//2, head-group g = core%2).
Each core computes 8 heads of one batch; the output projection partials
(row-parallel over the contracted dim) are summed on the host.

Per-core device program (all matmul operands bf16, fp32 PSUM accumulate):
  phase 1: qT = (Wq_g x_b)          [d=512, T] (transposed layout)
           kT likewise; v natural [T, d] packed into v_aug [j, head, 65]
           with a ones column (col 64) appended per head.
  phase 2: per (i-chunk 512, head-pair):
           sT[j,i] = kT.T qT via two row-packed K=64 matmuls
           e = exp(sT/32)                      (ScalarE, from PSUM)
           g = (e - 1) * mask                  (one fused DVE op, bf16 4x)
           out[65, i] += v_aug.T g             (accumulate over j)
         Using f = mask ? e : 1 = g + 1, so f@v = g@v + sum_t(v) and the
         ones column makes row 64 the softmax denominator minus T.
         f_out = out + v1 (host-precomputed [sum_t v; T]) then rows 0..63
         are scaled by 1/row64.
  phase 3: yT_partial[c,t] = Wp_g.T x_att  -> fp32 out, host adds the two
           group partials, transposes, adds bias.
"""

import sys

sys.path.insert(0, "/opt/trn_rl_repo")

from contextlib import ExitStack

import numpy as np
import ml_dtypes

import concourse.bass as bass  # noqa: F401  (import keeps bass registered)
import concourse.mybir as mybir
import concourse.tile as tile
from concourse import bacc
from concourse.bass_utils import run_bass_kernel_spmd

B, T, C, H = 4, 2048, 1024, 16
HD = C // H  # 64
NCORE = 8
DG = C // 2  # dims per core = 512 (8 heads)
HG = H // 2  # heads per core = 8
SCALE = float(C) ** -0.5

BF16 = mybir.dt.bfloat16
F32 = mybir.dt.float32
bf = ml_dtypes.bfloat16
AF = mybir.ActivationFunctionType
ALU = mybir.AluOpType

_CACHE = {}


def build_nc(t=T):
    """Build + compile the SPMD program for sequence length t (t % 512 == 0)."""
    nT4 = t // 512  # 512-wide i/t chunks
    nT16 = t // 128  # 128-wide j/t chunks

    nc = bacc.Bacc("TRN2", target_bir_lowering=False, debug=False, num_devices=NCORE)

    xq = nc.dram_tensor("xq", [C, t], BF16, kind="ExternalInput")
    xk = nc.dram_tensor("xk", [C, t], BF16, kind="ExternalInput")
    xv = nc.dram_tensor("xv", [C, t], BF16, kind="ExternalInput")
    mt = nc.dram_tensor("mt", [t, t], BF16, kind="ExternalInput")
    wq = nc.dram_tensor("wq", [C, DG], BF16, kind="ExternalInput")
    wk = nc.dram_tensor("wk", [C, DG], BF16, kind="ExternalInput")
    wv = nc.dram_tensor("wv", [C, DG], BF16, kind="ExternalInput")
    wp = nc.dram_tensor("wp", [DG, C], BF16, kind="ExternalInput")
    v1 = nc.dram_tensor("v1", [HD + 1, HG], F32, kind="ExternalInput")
    yt = nc.dram_tensor("yt", [C, t], F32, kind="ExternalOutput")

    xq_v = xq.rearrange("(cc p) t -> p cc t", p=128)
    xk_v = xk.rearrange("(cc p) t -> p cc t", p=128)
    xv_v = xv.rearrange("(cc p) t -> p cc t", p=128)
    mt_v = mt.rearrange("(jc p) i -> p jc i", p=128)
    yt_v = yt.rearrange("(cc p) t -> p cc t", p=128)

    with tile.TileContext(nc) as tc, ExitStack() as ctx:
        consts = ctx.enter_context(tc.tile_pool(name="consts", bufs=1))
        qk = ctx.enter_context(tc.tile_pool(name="qk", bufs=1))
        vap = ctx.enter_context(tc.tile_pool(name="vap", bufs=1))
        mpool = ctx.enter_context(tc.tile_pool(name="mask", bufs=2))
        wpool = ctx.enter_context(tc.tile_pool(name="wqkv", bufs=1))
        xin = ctx.enter_context(tc.tile_pool(name="xin", bufs=3))
        epool = ctx.enter_context(tc.tile_pool(name="e", bufs=6))
        xatt = ctx.enter_context(tc.tile_pool(name="xatt", bufs=1))
        fpool = ctx.enter_context(tc.tile_pool(name="fz", bufs=2))
        rpool = ctx.enter_context(tc.tile_pool(name="rz", bufs=2))
        ypool = ctx.enter_context(tc.tile_pool(name="yout", bufs=2))
        ps_p = ctx.enter_context(tc.tile_pool(name="ps_p", bufs=2, space="PSUM"))
        ps_o = ctx.enter_context(tc.tile_pool(name="ps_o", bufs=2, space="PSUM"))
        ps_b = ctx.enter_context(tc.tile_pool(name="ps_b", bufs=2, space="PSUM"))

        # Mask prefetch: first two i-chunk slices queued before everything.
        mt_tiles = {}

        def load_mask(i4):
            mt_sb = mpool.tile([128, nT16, 512], BF16, tag="mask")
            nc.sync.dma_start(out=mt_sb, in_=mt_v[:, :, i4 * 512 : (i4 + 1) * 512])
            mt_tiles[i4] = mt_sb

        v1_sb = consts.tile([HD + 1, HG], F32)
        nc.sync.dma_start(out=v1_sb, in_=v1[:, :])

        qT_sb = qk.tile([128, 4, t], BF16)
        kT_sb = qk.tile([128, 4, t], BF16)
        v_aug = vap.tile([128, nT16, HG, HD + 1], BF16)
        nc.vector.memset(v_aug, 1.0)

        wq_sb = wpool.tile([128, 8, DG], BF16)
        nc.sync.dma_start(out=wq_sb, in_=wq.rearrange("(cc p) d -> p cc d", p=128))
        wk_sb = wpool.tile([128, 8, DG], BF16)
        nc.sync.dma_start(out=wk_sb, in_=wk.rearrange("(cc p) d -> p cc d", p=128))
        wv_sb = wpool.tile([128, 8, DG], BF16)
        nc.sync.dma_start(out=wv_sb, in_=wv.rearrange("(cc p) d -> p cc d", p=128))

        def qk_unit(which, dc, t4):
            # x streamed per (proj, t4) tile; x is re-read from HBM for each
            # d-chunk, trading DMA volume for SBUF residency.
            w_sb, x_v, out_sb = (
                (wq_sb, xq_v, qT_sb) if which == "q" else (wk_sb, xk_v, kT_sb)
            )
            x_sb = xin.tile([128, 8, 512], BF16, tag="xin")
            nc.sync.dma_start(out=x_sb, in_=x_v[:, :, t4 * 512 : (t4 + 1) * 512])
            ps = ps_p.tile([128, 512], F32, tag="ps_p")
            for cc in range(8):
                nc.tensor.matmul(
                    ps,
                    lhsT=w_sb[:, cc, dc * 128 : (dc + 1) * 128],
                    rhs=x_sb[:, cc, :],
                    start=(cc == 0),
                    stop=(cc == 7),
                )
            nc.vector.tensor_copy(
                out=out_sb[:, dc, t4 * 512 : (t4 + 1) * 512], in_=ps
            )

        def qk_chunk(dc):
            for which in ("q", "k"):
                for t4 in range(nT4):
                    qk_unit(which, dc, t4)

        def v_chunk(tq):
            xv_sb = xin.tile([128, 8, 512], BF16, tag="xin")
            nc.sync.dma_start(out=xv_sb, in_=xv_v[:, :, tq * 512 : (tq + 1) * 512])
            for ts4 in range(4):
                t16 = tq * 4 + ts4
                ps = ps_p.tile([128, 512], F32, tag="ps_p")
                for cc in range(8):
                    nc.tensor.matmul(
                        ps,
                        lhsT=xv_sb[:, cc, ts4 * 128 : (ts4 + 1) * 128],
                        rhs=wv_sb[:, cc, :],
                        start=(cc == 0),
                        stop=(cc == 7),
                    )
                nc.vector.tensor_copy(
                    out=v_aug[:, t16, :, 0:HD],
                    in_=ps.rearrange("p (h d) -> p h d", h=HG),
                )

        def attention_pair(i4, p, fillers=()):
            fillers = list(fillers)
            isl = slice(i4 * 512, (i4 + 1) * 512)
            mt_sb = mt_tiles[i4]
            hA, hB = 2 * p, 2 * p + 1
            oA = ps_o.tile([HD + 1, 512], F32, tag="ps_o")
            oB = ps_o.tile([HD + 1, 512], F32, tag="ps_o")
            for jj in range(nT16 // 2):
                # e_big: exp for two j-blocks of this head pair,
                # laid out [j0-A | j0-B | j1-A | j1-B], 512 cols each.
                e_big = epool.tile([128, 2, 2, 512], BF16, tag="e")
                for dj in range(2):
                    j = 2 * jj + dj
                    jsl = slice(j * 128, (j + 1) * 128)
                    s_pair = ps_b.tile([128, 1024], F32, tag="s_pair")
                    nc.tensor.matmul(
                        s_pair[:, 0:512],
                        lhsT=kT_sb[0:64, p, jsl],
                        rhs=qT_sb[0:64, p, isl],
                        start=True,
                        stop=True,
                        tile_position=(0, 0),
                    )
                    nc.tensor.matmul(
                        s_pair[:, 512:1024],
                        lhsT=kT_sb[64:128, p, jsl],
                        rhs=qT_sb[64:128, p, isl],
                        start=True,
                        stop=True,
                        tile_position=(64, 0),
                    )
                    nc.scalar.activation(
                        out=e_big[:, dj, :, :], in_=s_pair, func=AF.Exp, scale=SCALE
                    )
                # e -= 1 (TensorScalar, 4x) then e *= m (TensorTensor, 2x),
                # both in place; mask broadcast over the head dim.
                nc.vector.tensor_scalar(
                    out=e_big, in0=e_big, scalar1=1.0, scalar2=None, op0=ALU.subtract
                )
                nc.vector.tensor_mul(
                    e_big,
                    e_big,
                    mt_sb[:, 2 * jj : 2 * jj + 2, None, :].broadcast_to(
                        [128, 2, 2, 512]
                    ),
                )
                for dj in range(2):
                    j = 2 * jj + dj
                    nc.tensor.matmul(
                        oA,
                        lhsT=v_aug[:, j, hA, :],
                        rhs=e_big[:, dj, 0, :],
                        start=(j == 0),
                        stop=(j == nT16 - 1),
                    )
                    nc.tensor.matmul(
                        oB,
                        lhsT=v_aug[:, j, hB, :],
                        rhs=e_big[:, dj, 1, :],
                        start=(j == 0),
                        stop=(j == nT16 - 1),
                    )
                if fillers:
                    fillers.pop(0)()
            for h, o_ps in ((hA, oA), (hB, oB)):
                f_sb = fpool.tile([HD + 1, 512], F32, tag="fz")
                nc.vector.tensor_scalar(
                    out=f_sb,
                    in0=o_ps,
                    scalar1=v1_sb[:, h : h + 1],
                    scalar2=None,
                    op0=ALU.add,
                )
                rz = rpool.tile([1, 512], F32, tag="rz")
                nc.vector.reciprocal(rz, f_sb[HD : HD + 1, :])
                rb = rpool.tile([64, 512], F32, tag="rb")
                nc.gpsimd.partition_broadcast(rb, rz)
                po = (h % 2) * 64
                nc.gpsimd.tensor_mul(xatt_sb[po : po + 64, p, isl], f_sb[0:HD, :], rb)

        def proj_unit(t4, cc):
            ps = ps_p.tile([128, 512], F32, tag="ps_p")
            for dc in range(4):
                nc.tensor.matmul(
                    ps,
                    lhsT=wp_sb[:, dc, cc * 128 : (cc + 1) * 128],
                    rhs=xatt_sb[:, dc, t4 * 512 : (t4 + 1) * 512],
                    start=(dc == 0),
                    stop=(dc == 3),
                )
            y_sb = ypool.tile([128, 512], F32, tag="y")
            nc.vector.tensor_copy(out=y_sb, in_=ps)
            nc.sync.dma_start(out=yt_v[:, cc, t4 * 512 : (t4 + 1) * 512], in_=y_sb)

        def proj_chunk(t4):
            for cc in range(8):
                proj_unit(t4, cc)

        xatt_sb = xatt.tile([128, 4, t], BF16)

        # Woven schedule: qk d-chunk p+1 and output-projection units are
        # interleaved into the attention jj-loops as PE filler so ACT/DVE
        # pace the pipeline without serializing against projections.
        qk_chunk(0)
        v_chunk(0)
        load_mask(0)
        if nT4 > 1:
            load_mask(1)
        wp_sb = consts.tile([128, 4, C], BF16)
        nc.sync.dma_start(out=wp_sb, in_=wp.rearrange("(dc p) c -> p dc c", p=128))
        for tq in range(1, nT4):
            v_chunk(tq)
        for i4 in range(nT4):
            if i4 + 2 < nT4:
                load_mask(i4 + 2)
            for p in range(4):
                fillers = []
                if i4 == 0 and 0 < p < 3:
                    fillers += [
                        (lambda w, t4, dc=p + 1: (lambda: qk_unit(w, dc, t4)))(w, t4)
                        for w in ("q", "k")
                        for t4 in range(nT4)
                    ]
                if i4 == 0 and p == 0:
                    qk_chunk(1)
                if i4 > 0:
                    # two output-projection units per pair of the next i4
                    t4p = i4 - 1
                    fillers += [
                        (lambda cc: (lambda: proj_unit(t4p, cc)))(cc)
                        for cc in range(2 * p, 2 * p + 2)
                    ]
                attention_pair(i4, p, fillers)
        proj_chunk(nT4 - 1)

    nc.compile()
    return nc


def _prep_in_maps(query, key, value, mask, Wq, Wk, Wv, Wp):
    query = np.asarray(query, np.float32)
    key = np.asarray(key, np.float32)
    value = np.asarray(value, np.float32)
    mask2d = np.asarray(mask, np.int32).reshape(mask.shape[-2], mask.shape[-1])
    Wq = np.asarray(Wq, np.float32)
    Wk = np.asarray(Wk, np.float32)
    Wv = np.asarray(Wv, np.float32)
    Wp = np.asarray(Wp, np.float32)

    t = query.shape[1]
    mt_np = np.ascontiguousarray(mask2d.T).astype(bf)
    per_g = []
    for g in range(2):
        sl = slice(DG * g, DG * (g + 1))
        per_g.append(
            dict(
                wq=np.ascontiguousarray(Wq[sl, :].T).astype(bf),
                wk=np.ascontiguousarray(Wk[sl, :].T).astype(bf),
                wv=np.ascontiguousarray(Wv[sl, :].T).astype(bf),
                wp=np.ascontiguousarray(Wp[:, sl].T).astype(bf),
                Wv_f32=Wv[sl, :],
            )
        )
    in_maps = []
    for core in range(NCORE):
        b, g = core // 2, core % 2
        sv = value[b].sum(axis=0)  # [C]
        V1g = per_g[g]["Wv_f32"] @ sv  # [DG]
        v1_np = np.empty((HD + 1, HG), np.float32)
        v1_np[0:HD, :] = V1g.reshape(HG, HD).T
        v1_np[HD, :] = float(t)
        in_maps.append(
            dict(
                xq=np.ascontiguousarray(query[b].T).astype(bf),
                xk=np.ascontiguousarray(key[b].T).astype(bf),
                xv=np.ascontiguousarray(value[b].T).astype(bf),
                mt=mt_np,
                wq=per_g[g]["wq"],
                wk=per_g[g]["wk"],
                wv=per_g[g]["wv"],
                wp=per_g[g]["wp"],
                v1=v1_np,
            )
        )
    return in_maps


def kernel(query, key, value, mask, Wq, Wk, Wv, Wp, bp, **run_kwargs):
    if "nc" not in _CACHE:
        _CACHE["nc"] = build_nc(np.asarray(query).shape[1])
    nc = _CACHE["nc"]
    in_maps = _prep_in_maps(query, key, value, mask, Wq, Wk, Wv, Wp)
    res = run_bass_kernel_spmd(nc, in_maps, list(range(NCORE)), **run_kwargs)
    _CACHE["last_result"] = res
    bp = np.asarray(bp, np.float32)
    t = np.asarray(query).shape[1]
    y = np.empty((B, t, C), np.float32)
    for b in range(B):
        y_t = res.results[2 * b]["yt"] + res.results[2 * b + 1]["yt"]  # [C, t]
        y[b] = y_t.T + bp
    return y
